# revision 19
# baseline (speedup 1.0000x reference)
"""Trainium2 Bass kernel for nn_AttnBlock_16887811407979 (sparse attention).

Strategy: 8-way sequence-parallel SPMD (each core handles a 256-query
slice, all heads), no collectives. The sparse gather is densified: the
host converts (attendable_indices, valid_indices_mask) into a dense
count matrix C[n, q], so softmax-over-slots == count-weighted dense
softmax:
    W[n,q] = C[n,q] * exp(S^T[n,q]);  O[q] = (W^T V) / sum_n W[n,q].

v2 orchestration changes vs v1:
  - x shipped bf16 (halves the critical head DMA)
  - PE warm-up dummy matmuls bridge preamble->conv so HAM hits 8/8 early
  - single ACT table set (natural_log_exp): GN 1/sqrt(var) via
    exp(-0.5 ln var), softmax 1/s via exp(-ln s) -- kills the sqrt
    table load and the 4.4us single-partition DVE reciprocals
  - K bias dropped (softmax-invariant); Q/V biases folded into the
    conv matmul accumulation as rank-1 ones-row matmuls
  - O^T accumulators copied out of PSUM immediately (bf16), softmax
    normalization runs on SBUF overlapped with the next pass
  - proj split: pass-0 head blocks accumulate while pass-1 normalizes
"""
import sys
import types
import contextlib

sys.path.insert(0, '/opt/trn_rl_repo')
sys.path.insert(0, '/root/.axon_site')

import numpy as np
import ml_dtypes

import concourse.bass as bass
import concourse.tile as tile
from concourse import mybir
from concourse.vector_clock import ScopedClock
from concourse.bass_utils import run_bass_kernel_spmd

f32 = mybir.dt.float32
f32r = mybir.dt.float32r
bf16 = mybir.dt.bfloat16
AF = mybir.ActivationFunctionType
AX = mybir.AxisListType
ALU = mybir.AluOpType

N_CORES = 8
C = 512
N = 2048
HEADS = 8
D = 64
K_IDX = 128
GROUPS = 32
GSIZE = C // GROUPS          # 16 channels per group
NQ = N // N_CORES            # 256 queries per core
NCHUNK = N // 128            # 16 key chunks
CCHUNK = C // 128            # 4 channel chunks
EPS = 1e-6

N_WARM = 34                  # PE warm-up dummy matmuls (tunable)

# head -> block mapping: even heads (lhsT base partition 0) in even-bank
# score slots, odd heads in odd banks, so concurrently-issued row-group
# pairs never share a PSUM bank.
BLK = [4 * (h // 4) + (h % 4) // 2 + 2 * (h % 2) for h in range(HEADS)]
HB = [0] * 8
for _h in range(HEADS):
    HB[BLK[_h]] = _h                                     # b -> h

# ---------------------------------------------------------------------------
# walrus workaround: this container's walrus accepts at most ONE embedded
# sync-wait per engine instruction. Split Tile's multi-wait instructions
# into chains of single-wait NoOps, and do the same for the kernel-tail
# drain that Tile emits at TileContext exit.
# ---------------------------------------------------------------------------
_wsplit = [0]


def _drain_and_barrier_split(self, tick_clock, wait_clock):
    nc = self.nc
    carrier = nc.sync.nop(nofuse=True)
    wait_clock.add_sem_waits(
        carrier.ins, ScopedClock({None: tick_clock.global_clock}))
    si = carrier.ins.sync_info
    waits = list(si.on_wait or []) if si is not None else []
    if len(waits) > 1:
        carrier.ins.sync_info = mybir.SyncInfo(
            on_wait=waits[:1], on_update=list(si.on_update or []))
        for w in waits[1:]:
            extra = nc.sync.nop(nofuse=True)
            extra.ins.sync_info = mybir.SyncInfo(on_wait=[w], on_update=[])
    nc.sync.drain()
    nc.all_engine_barrier(sem_only=True)
    assert self.sems is not None
    popped = nc._tile_sem_poison_stack.pop()
    assert popped is self._sem_poison
    nc.clear_and_free_semaphores(list(self.sems.allocated().values()))
    nc.all_engine_barrier(sem_only=True)


def _split_sync_waits(nc, max_waits=1):
    for f in nc.m.functions:
        for bb in f.blocks:
            insts = bb.instructions
            out = []
            changed = False
            for inst in insts:
                si = inst.sync_info
                waits = list(si.on_wait or []) if si is not None else []
                if len(waits) > max_waits:
                    changed = True
                    for i in range(len(waits) - max_waits):
                        _wsplit[0] += 1
                        nop = mybir.InstNoOp(
                            name=f"I-wsplit-{_wsplit[0]}", ins=[], outs=[])
                        nop.engine = inst.engine
                        nop.sync_info = mybir.SyncInfo(
                            on_wait=[waits[i]], on_update=[])
                        out.append(nop)
                    inst.sync_info = mybir.SyncInfo(
                        on_wait=waits[len(waits) - max_waits:],
                        on_update=list(si.on_update or []))
                out.append(inst)
            if changed:
                if isinstance(insts, list):
                    insts[:] = out
                else:
                    bb.instructions = out


tile.TileContext._drain_and_barrier = _drain_and_barrier_split


# ---------------------------------------------------------------------------
# kernel builder
# ---------------------------------------------------------------------------

def _build(split_waits=True):
    nc = bass.Bass("TRN2", target_bir_lowering=False, debug=False)

    def din(name, shape, dt=f32):
        return nc.dram_tensor(name, shape, dt, kind="ExternalInput").ap()

    xbf_d = din("xbf", [C, N], bf16)
    xq_d = din("xq", [C, NQ])
    cnt_d = din("cnt", [N, NQ], bf16)
    wkT_d = din("wkT", [C, C])
    wqT_d = din("wqT", [C, C])
    wvT_d = din("wvT", [C, C])
    wpTb_d = din("wpTb", [C, C])
    smalls_d = din("smalls", [128, 20])
    brow_d = din("brow", [2, C])
    gind_d = din("gind", [128, 32 * CCHUNK])
    gindT_d = din("gindT", [GROUPS, C])
    out_d = nc.dram_tensor("out", [C, NQ], f32, kind="ExternalOutput").ap()
    dbg_d = nc.dram_tensor("dbg", [65, 2048], f32, kind="ExternalOutput").ap()
    dbg2_d = nc.dram_tensor("dbg2", [64, 2048], f32, kind="ExternalOutput").ap()

    with tile.TileContext(nc) as tc, contextlib.ExitStack() as ctx:
        P = ctx.enter_context(tc.tile_pool(name="persist", bufs=1))
        A = ctx.enter_context(tc.tile_pool(name="phase_a", bufs=1))

        # ---- early DMAs: x first (GN stats are the critical path) ----
        xt = [A.tile([128, N], bf16, tag=f"xt{k}", name=f"xt{k}")
              for k in range(CCHUNK)]
        for k in range(CCHUNK):
            nc.gpsimd.dma_start(xt[k][:], xbf_d[128 * k:128 * (k + 1), :])
        smallst = P.tile([128, 20], f32, tag="smalls", name="smalls")
        nc.sync.dma_start(smallst[:], smalls_d)
        gindt = P.tile([128, 32 * CCHUNK], f32, tag="gind", name="gind")
        nc.sync.dma_start(gindt[:], gind_d)
        gindTt = P.tile([GROUPS, C], f32, tag="gindT", name="gindT")
        nc.sync.dma_start(gindTt[:], gindT_d)
        bqrow_t = P.tile([1, C], f32, tag="bqrow", name="bqrow")
        nc.sync.dma_start(bqrow_t[:], brow_d[0:1, :])
        bvrow_t = P.tile([1, C], f32, tag="bvrow", name="bvrow")
        nc.sync.dma_start(bvrow_t[:], brow_d[1:2, :])
        xqt = [P.tile([128, NQ], f32, tag=f"xqt{k}", name=f"xqt{k}")
               for k in range(CCHUNK)]
        for k in range(CCHUNK):
            nc.sync.dma_start(xqt[k][:], xq_d[128 * k:128 * (k + 1), :])

        def sm(k, f):
            return smallst[:, 5 * k + f:5 * k + f + 1]

        # ---- PE warm-up: dummy matmuls so HAM reaches 8/8 before the
        # convs; they have no consumers and burn ~220ns each warm.
        junk = P.tile([128, 640], bf16, tag="junk", name="junk")
        nc.gpsimd.memset(junk[:], 0.125)
        ones1 = P.tile([1, C], f32, tag="ones1", name="ones1")
        nc.vector.memset(ones1[:], 1.0)
        warm_cm = tc.tile_pool(name="warm", bufs=1, space="PSUM")
        warm = warm_cm.__enter__()
        wps = warm.tile([128, 512], f32, tag="wps", name="wps")
        for i in range(N_WARM):
            nc.tensor.matmul(wps[:], junk[:, 0:128], junk[:, 128:640],
                             start=True, stop=True)

        # ---- GroupNorm stats, pipelined per x chunk ----
        s1 = [P.tile([128, 1], f32, tag=f"s1{k}", name=f"s1{k}")
              for k in range(CCHUNK)]
        s2 = [P.tile([128, 1], f32, tag=f"s2{k}", name=f"s2{k}")
              for k in range(CCHUNK)]
        At = [P.tile([128, 1], f32, tag=f"A{k}", name=f"A{k}")
              for k in range(CCHUNK)]
        Bt = [P.tile([128, 1], f32, tag=f"B{k}", name=f"B{k}")
              for k in range(CCHUNK)]
        sq = A.tile([128, N], bf16, tag="sq", name="sq")
        with tc.tile_pool(name="gnps", bufs=1, space="PSUM") as gnps:
            for k in range(CCHUNK):
                nc.vector.tensor_reduce(s1[k][:], xt[k][:],
                                        axis=AX.X, op=ALU.add)
                nc.scalar.activation(sq[:], xt[k][:], AF.Square,
                                     accum_out=s2[k][:])
            gs = gnps.tile([GROUPS, 2], f32, tag="gs", name="gs")
            for k in range(CCHUNK):
                nc.tensor.matmul(gs[:, 0:1], gindt[:, 32 * k:32 * (k + 1)],
                                 s1[k][:], start=(k == 0),
                                 stop=(k == CCHUNK - 1))
            for k in range(CCHUNK):
                nc.tensor.matmul(gs[:, 1:2], gindt[:, 32 * k:32 * (k + 1)],
                                 s2[k][:], start=(k == 0),
                                 stop=(k == CCHUNK - 1))
            mstat = P.tile([GROUPS, 2], f32, tag="mstat", name="mstat")
            inv_n = 1.0 / (GSIZE * N)
            nc.vector.tensor_scalar_mul(mstat[:, 0:1], gs[:, 0:1], inv_n)
            msq = P.tile([GROUPS, 1], f32, tag="msq", name="msq")
            nc.vector.tensor_scalar_mul(msq[:], gs[:, 1:2], inv_n)
            m2 = P.tile([GROUPS, 1], f32, tag="m2", name="m2")
            nc.vector.tensor_mul(m2[:], mstat[:, 0:1], mstat[:, 0:1])
            var = P.tile([GROUPS, 1], f32, tag="var", name="var")
            nc.vector.tensor_sub(var[:], msq[:], m2[:])
            nc.vector.tensor_scalar_add(var[:], var[:], float(EPS))
            # 1/sqrt(var) = exp(-0.5 ln var): stays in the exp table set
            lv = P.tile([GROUPS, 1], f32, tag="lv", name="lv")
            nc.scalar.activation(lv[:], var[:], AF.Ln)
            nc.scalar.activation(mstat[:, 1:2], lv[:], AF.Exp, scale=-0.5)
            mr = [P.tile([128, 2], f32, tag=f"mr{k}", name=f"mr{k}")
                  for k in range(CCHUNK)]
            for k in range(CCHUNK):
                mrp = gnps.tile([128, 2], f32, tag="mrp", name="mrp", bufs=2)
                nc.tensor.matmul(mrp[:], gindTt[:, 128 * k:128 * (k + 1)],
                                 mstat[:], start=True, stop=True)
                nc.vector.tensor_copy(mr[k][:], mrp[:])
            for k in range(CCHUNK):
                nc.vector.tensor_mul(At[k][:], sm(k, 3), mr[k][:, 1:2])
                tmp = P.tile([128, 1], f32, tag="tmpB", name="tmpB")
                nc.vector.tensor_mul(tmp[:], mr[k][:, 0:1], At[k][:])
                nc.vector.tensor_sub(Bt[k][:], sm(k, 4), tmp[:])
        warm_cm.__exit__(None, None, None)

        # ---- affine: hq (query slice, f32 in) then h (full, bf16 in) ----
        hqt = [P.tile([128, NQ], f32r, tag=f"hqt{k}", name=f"hqt{k}")
               for k in range(CCHUNK)]
        ht = [A.tile([128, N], f32r, tag=f"ht{k}", name=f"ht{k}")
              for k in range(CCHUNK)]
        with nc.allow_low_precision(reason="f32r affine"):
            for k in range(CCHUNK):
                nc.vector.tensor_scalar(
                    hqt[k][:], xqt[k][:], At[k][:, 0:1], Bt[k][:, 0:1],
                    op0=ALU.mult, op1=ALU.add)
            for g in range(4):
                cols = slice(512 * g, 512 * (g + 1))
                for k in range(CCHUNK):
                    nc.vector.tensor_scalar(
                        ht[k][:, cols], xt[k][:, cols],
                        At[k][:, 0:1], Bt[k][:, 0:1],
                        op0=ALU.mult, op1=ALU.add)

        # ---- weight DMAs (behind x in queue order) ----
        wq = [A.tile([128, C], f32r, tag=f"wq{k}", name=f"wq{k}")
              for k in range(CCHUNK)]
        wk = [A.tile([128, C], f32r, tag=f"wk{k}", name=f"wk{k}")
              for k in range(CCHUNK)]
        wv = [A.tile([128, C], f32r, tag=f"wv{k}", name=f"wv{k}")
              for k in range(CCHUNK)]
        wpb = [P.tile([64, C], bf16, tag=f"wpb{b}", name=f"wpb{b}")
               for b in range(HEADS)]
        for k in range(CCHUNK):
            r = slice(128 * k, 128 * (k + 1))
            nc.gpsimd.dma_start(wq[k][:], wqT_d[r, :])
        for k in range(CCHUNK):
            r = slice(128 * k, 128 * (k + 1))
            nc.gpsimd.dma_start(wk[k][:], wkT_d[r, :])
        for k in range(CCHUNK):
            r = slice(128 * k, 128 * (k + 1))
            nc.gpsimd.dma_start(wv[k][:], wvT_d[r, :])
        cntt = P.tile([128, 256 * NCHUNK], bf16, tag="cntt", name="cntt")
        nc.sync.dma_start(
            cntt[:].rearrange("p (m q) -> p m q", m=NCHUNK),
            cnt_d.rearrange("(m p) q -> p m q", p=128))
        for b in range(HEADS):
            nc.gpsimd.dma_start(wpb[b][:], wpTb_d[64 * b:64 * (b + 1), :])

        kt = [P.tile([128, N], f32r, tag=f"kt{k}", name=f"kt{k}")
              for k in range(CCHUNK)]
        qt = [P.tile([128, NQ], f32r, tag=f"qt{k}", name=f"qt{k}")
              for k in range(CCHUNK)]
        vt = [P.tile([128, 65 * HEADS], bf16, tag=f"vt{m}", name=f"vt{m}")
              for m in range(NCHUNK)]
        on = P.tile([64, 256 * HEADS], bf16, tag="on", name="on")
        oraw = [P.tile([65, 256 * 4], bf16, tag=f"oraw{p}", name=f"oraw{p}")
                for p in range(2)]
        rr = [P.tile([1, 256 * 4], bf16, tag=f"rr{p}", name=f"rr{p}")
              for p in range(2)]
        rb = [P.tile([64, 256 * 4], bf16, tag=f"rb{p}", name=f"rb{p}")
              for p in range(2)]
        lnt = [P.tile([1, 256 * 4], f32, tag=f"lnt{p}", name=f"lnt{p}")
               for p in range(2)]
        onesr = P.tile([1, C], f32, tag="onesr", name="onesr")
        nc.vector.memset(onesr[:], 1.0)
        onesb = P.tile([1, 64], bf16, tag="onesb", name="onesb")
        nc.gpsimd.memset(onesb[:], 1.0)
        nbias = P.tile([1, 1], f32, tag="nbias", name="nbias")
        nc.gpsimd.memset(nbias[:], float(-32.0 * np.log(2.0)))

        # ---- dense conv phase ----
        with tc.tile_pool(name="asb", bufs=2) as asb:
            cps_cm = tc.tile_pool(name="cps", bufs=4, space="PSUM")
            cps = cps_cm.__enter__()

            # Q conv (+bq via ones-row matmul); K bias is softmax-invariant
            # and dropped entirely.
            for m in range(CCHUNK):
                pq = cps.tile([128, 512], f32, tag="cp", name="cpq")[:, 0:NQ]
                nc.tensor.matmul(pq[:], bqrow_t[0:1, 128 * m:128 * (m + 1)],
                                 onesr[0:1, 0:NQ], start=True, stop=False)
                for ci in range(CCHUNK):
                    nc.tensor.matmul(pq[:], wq[ci][:, 128 * m:128 * (m + 1)],
                                     hqt[ci][:],
                                     start=False, stop=(ci == CCHUNK - 1))
                nc.vector.tensor_copy(qt[m][:], pq[:])

            def kconv_colgroup(j):
                cols = slice(512 * j, 512 * (j + 1))
                for m in range(CCHUNK):
                    pk = cps.tile([128, 512], f32, tag="cp", name="cpk")
                    for ci in range(CCHUNK):
                        nc.tensor.matmul(
                            pk[:], wk[ci][:, 128 * m:128 * (m + 1)],
                            ht[ci][:, cols],
                            start=(ci == 0), stop=(ci == CCHUNK - 1))
                    nc.vector.tensor_copy(kt[m][:, cols], pk[:])

            def vconv(m):
                pv = cps.tile([128, C], f32, tag="cp", name="cpv")
                nc.tensor.matmul(pv[:], onesr[0:1, 0:128],
                                 bvrow_t[0:1, :],
                                 start=True, stop=False)
                for ci in range(CCHUNK):
                    nc.tensor.matmul(pv[:],
                                     ht[ci][:, 128 * m:128 * (m + 1)],
                                     wv[ci][:], start=False,
                                     stop=(ci == CCHUNK - 1))
                dst = vt[m][:].rearrange("p (h e) -> p h e", h=HEADS)[:, :, 0:64]
                nc.vector.tensor_copy(
                    dst, pv[:].rearrange("p (h d) -> p h d", h=HEADS))
                ones_cols = vt[m][:].rearrange(
                    "p (h e) -> p h e", h=HEADS)[:, :, 64:65]
                nc.gpsimd.memset(ones_cols, 1.0)

            def attn_scores(p, m):
                heads = range(4 * p, 4 * p + 4)
                st = sps.tile([128, 256 * 4], f32, tag="st", name=f"st{p}_{m}")
                for h in heads:
                    par = h % 2
                    cm = h // 2
                    lb = BLK[h] - 4 * p
                    nc.tensor.matmul(
                        st[:, 256 * lb:256 * (lb + 1)],
                        kt[cm][64 * par:64 * (par + 1),
                               128 * m:128 * (m + 1)],
                        qt[cm][64 * par:64 * (par + 1), :],
                        start=True, stop=True)
                et = asb.tile([128, 256 * 4], bf16, tag="et", name=f"et{p}_{m}")
                nc.scalar.activation(et[:], st[:], AF.Exp)
                wt = asb.tile([128, 256 * 4], bf16, tag="wt", name=f"wt{p}_{m}")
                nc.vector.tensor_mul(
                    wt[:].rearrange("p (b q) -> p b q", b=4),
                    et[:].rearrange("p (b q) -> p b q", b=4),
                    cntt[:, 256 * m:256 * (m + 1)].unsqueeze(1)
                        .broadcast_to([128, 4, NQ]))
                return wt

            def attn_ov(p, m, ot, wt):
                for h in range(4 * p, 4 * p + 4):
                    lb = BLK[h] - 4 * p
                    nc.tensor.matmul(
                        ot[0:65, 512 * lb:512 * lb + 256],
                        vt[m][:, 65 * h:65 * h + 65],
                        wt[:, 256 * lb:256 * (lb + 1)],
                        start=(m == 0), stop=(m == NCHUNK - 1))

            def norm_copy(p, ot):
                # free the PSUM accumulator fast: strided copy -> SBUF bf16
                nc.vector.tensor_copy(
                    oraw[p][:].rearrange("p (b q) -> p b q", b=4),
                    ot[0:65, :].rearrange("p (b w q) -> p b w q",
                                          b=4, w=2)[:, :, 0, :])

            def norm_math(p):
                # 1/s = exp(-ln s); broadcast across the 64 d-partitions
                # via a rank-1 PE matmul into a borrowed score-pool tile
                # ACT Ln is only accurate for |ln x| < ~40; pre-scale by
                # 2^-32 (free activation scale) and compensate in the Exp
                # bias: 1/s = exp(-ln(s 2^-32) - 32 ln 2).
                nc.scalar.activation(lnt[p][:], oraw[p][64:65, :], AF.Ln,
                                     scale=float(2.0 ** -32))
                nc.scalar.activation(rr[p][:], lnt[p][:], AF.Exp, scale=-1.0,
                                     bias=nbias[:, 0:1])
                rbp = sps.tile([128, 256 * 4], f32, tag="st",
                               name=f"rbp{p}")[0:64, :]
                for j in range(2):
                    cols = slice(512 * j, 512 * (j + 1))
                    nc.tensor.matmul(rbp[:, cols], onesb[:], rr[p][:, cols],
                                     start=True, stop=True)
                nc.vector.tensor_copy(rb[p][:], rbp[:])

            def norm_mul(p):
                nc.vector.tensor_mul(
                    on[:, 1024 * p:1024 * (p + 1)],
                    oraw[p][0:64, :], rb[p][:])

            # dense conv burst
            for j in range(4):
                kconv_colgroup(j)
                for m in range(4 * j, 4 * j + 4):
                    vconv(m)
            cps_cm.__exit__(None, None, None)

            # attention: st double-buffered (2x2 banks) + ot (4 banks)
            with tc.tile_pool(name="ops", bufs=1, space="PSUM") as ops, \
                    tc.tile_pool(name="sps", bufs=2, space="PSUM") as sps:
                ot0 = ops.tile([65, 512 * 4], f32, tag="ot", name="ot0")
                prev = None
                for m in range(NCHUNK):
                    wt = attn_scores(0, m)
                    if prev is not None:
                        attn_ov(0, m - 1, ot0, prev)
                    prev = wt
                attn_ov(0, NCHUNK - 1, ot0, prev)
                norm_copy(0, ot0)
                ot1 = ops.tile([65, 512 * 4], f32, tag="ot", name="ot1")
                prev = None
                for m in range(NCHUNK):
                    wt = attn_scores(1, m)
                    if prev is not None:
                        attn_ov(1, m - 1, ot1, prev)
                    prev = wt
                    if m == 2:
                        norm_math(0)
                    elif m == 4:
                        norm_mul(0)
                attn_ov(1, NCHUNK - 1, ot1, prev)
                norm_copy(1, ot1)
                norm_math(1)

        # ---- projection + residual ----
        # pass-0 head blocks (b 0..3) can accumulate while pass-1
        # normalization finishes; b 4..7 wait on norm_mul(1).
        with tc.tile_pool(name="pps", bufs=1, space="PSUM") as pps, \
                tc.tile_pool(name="psb", bufs=2) as psb:
            pj = []
            for m in range(CCHUNK):
                pjm = pps.tile([128, NQ], f32, tag=f"pj{m}", name=f"pj{m}")
                pj.append(pjm)
                for b in range(4):
                    nc.tensor.matmul(pjm[:],
                                     wpb[b][:, 128 * m:128 * (m + 1)],
                                     on[:, 256 * b:256 * (b + 1)],
                                     start=(b == 0), stop=False)
            norm_mul(1)
            for m in range(CCHUNK):
                for b in range(4, HEADS):
                    nc.tensor.matmul(pj[m][:],
                                     wpb[b][:, 128 * m:128 * (m + 1)],
                                     on[:, 256 * b:256 * (b + 1)],
                                     start=False, stop=(b == HEADS - 1))
                t1 = psb.tile([128, NQ], f32, tag="t1", name=f"t1{m}")
                nc.scalar.activation(t1[:], pj[m][:], AF.Identity,
                                     bias=sm(m, 2))
                outm = psb.tile([128, NQ], f32, tag="outm", name=f"outm{m}")
                nc.vector.tensor_add(outm[:], t1[:], xqt[m][:])
                nc.sync.dma_start(out_d[128 * m:128 * (m + 1), :], outm[:])
            for p in range(2):
                nc.gpsimd.dma_start(dbg_d[0:1, 1024 * p:1024 * (p + 1)],
                                    rr[p][:])
                nc.gpsimd.dma_start(dbg_d[1:2, 1024 * p:1024 * (p + 1)],
                                    lnt[p][:])
                nc.gpsimd.dma_start(dbg_d[2:3, 1024 * p:1024 * (p + 1)],
                                    oraw[p][64:65, :])
            nc.gpsimd.dma_start(dbg2_d[:, :], on[:])

    if split_waits:
        _split_sync_waits(nc)
    return nc


# ---------------------------------------------------------------------------
# host-side input prep + entry point
# ---------------------------------------------------------------------------

def _prep_inputs(x, valid_indices_mask, attendable_indices, gn_w, gn_b,
                 wq_, bq_, wk_, bk_, wv_, bv_, wp_, bp_):
    x = np.asarray(x, np.float32).reshape(C, N)
    idx = np.asarray(attendable_indices, np.int64)
    val = np.asarray(valid_indices_mask, np.float32)
    cnt_qn = np.zeros((N, N), np.float32)       # [q, n]
    rows = np.repeat(np.arange(N), K_IDX)
    np.add.at(cnt_qn, (rows, idx.reshape(-1)), val.reshape(-1))
    cntT = np.ascontiguousarray(cnt_qn.T).astype(ml_dtypes.bfloat16)  # [n, q]

    wq_ = np.asarray(wq_, np.float32)
    wk_ = np.asarray(wk_, np.float32)
    wv_ = np.asarray(wv_, np.float32)
    wp_ = np.asarray(wp_, np.float32)
    # wp column for o-channel (d*HEADS + h); our block order stacks head
    # HB[b] rows d-major at 64*b
    wpT = wp_.T                                    # [cin = d*8+h, cout]
    wpTb = np.empty((C, C), np.float32)
    for b in range(HEADS):
        h = HB[b]
        wpTb[64 * b:64 * (b + 1), :] = wpT[h::HEADS, :]   # d-major rows of head h

    gind = np.zeros((C, GROUPS), np.float32)
    gind[np.arange(C), np.arange(C) // GSIZE] = 1.0

    smalls = np.zeros((128, 20), np.float32)
    fields = [np.asarray(bk_, np.float32), np.asarray(bq_, np.float32),
              np.asarray(bp_, np.float32), np.asarray(gn_w, np.float32),
              np.asarray(gn_b, np.float32)]
    for k in range(CCHUNK):
        for f, arr in enumerate(fields):
            smalls[:, 5 * k + f] = arr.reshape(C)[128 * k:128 * (k + 1)]
    gind_all = np.zeros((128, 32 * CCHUNK), np.float32)
    for k in range(CCHUNK):
        gind_all[:, 32 * k:32 * (k + 1)] = gind[128 * k:128 * (k + 1), :]
    brow = np.stack([np.asarray(bq_, np.float32).reshape(C),
                     np.asarray(bv_, np.float32).reshape(C)])
    common = {
        "xbf": x.astype(ml_dtypes.bfloat16),
        "wkT": np.ascontiguousarray(wk_.T),
        "wqT": np.ascontiguousarray(wq_.T),
        "wvT": np.ascontiguousarray(wv_.T),
        "wpTb": wpTb,
        "smalls": smalls,
        "brow": brow,
        "gind": gind_all,
        "gindT": np.ascontiguousarray(gind.T),
    }
    in_maps = []
    for c in range(N_CORES):
        cols = slice(NQ * c, NQ * (c + 1))
        m = dict(common)
        m["xq"] = np.ascontiguousarray(x[:, cols])
        m["cnt"] = np.ascontiguousarray(cntT[:, cols])
        in_maps.append(m)
    return in_maps


def _enable_profile_hook():
    """Register the axon NTFF hook (this container's antenv lacks it)."""
    import antenv
    if 'antenv.axon_hooks' not in sys.modules:
        mod = types.ModuleType('antenv.axon_hooks')
        mod._hook = None
        mod.set_axon_ntff_profile_hook = lambda h: setattr(mod, '_hook', h)
        mod.get_axon_ntff_profile_hook = lambda: mod._hook
        sys.modules['antenv.axon_hooks'] = mod
        antenv.axon_hooks = mod
    from trn_agent_boot.trn_boot import _ntff_profile_via_ctypes
    sys.modules['antenv.axon_hooks'].set_axon_ntff_profile_hook(
        _ntff_profile_via_ctypes('/opt/axon/libaxon_pjrt.so'))
    import concourse.bass_utils as bu
    bu.upload_artifacts = lambda tmpdir: tmpdir


_CACHE = {}


def _run(inputs, trace=False):
    if "nc" not in _CACHE:
        _CACHE["nc"] = _build()
    nc = _CACHE["nc"]
    in_maps = _prep_inputs(
        inputs['x'], inputs['valid_indices_mask'],
        inputs['attendable_indices'], inputs['gn_w'], inputs['gn_b'],
        inputs['wq'], inputs['bq'], inputs['wk'], inputs['bk'],
        inputs['wv'], inputs['bv'], inputs['wp'], inputs['bp'])
    if trace:
        _enable_profile_hook()
    res = run_bass_kernel_spmd(nc, in_maps, list(range(N_CORES)), trace=trace)
    out = np.concatenate([res.results[c]["out"] for c in range(N_CORES)],
                         axis=1).reshape(1, C, N).astype(np.float32)
    return out, res


def kernel(**inputs):
    out, _ = _run(inputs, trace=False)
    return out
